# revision 16
# baseline (speedup 1.0000x reference)
"""Trainium2 Bass kernel for the DSS (Diagonal State Space) layer.

y = irfft(rfft(u, 2L) * rfft(K, 2L))[:L] + D*u, with K the length-L DSS kernel
derived from (Lambda, W, log_step) via a complex softmax.

Strategy (v5, fp8, 384-step chunks):
  - The D*u term carries ~97% of the output energy for the reference params;
    it is added EXACTLY on the host.  The device computes only the small
    convolution part, which tolerates fp8 noise easily.
  - fp8 e4m3 on the wire: u in (2.1 MB/core), y_conv out (2.1 MB/core).
  - Chunked diagonal-SSM scan, time-major, 384-step chunks (3 blocks of 128):
    intra-chunk Toeplitz matmuls + rank-128 state (Re/Im of 64 modes).
    Per full chunk only 7 fp8 DoubleRow-packed matmuls (5 psY + 2 psS),
    using the (A0|M)@(u0|S) pairing so the state transition rides along
    with a u contraction.  72 matmuls total vs 80 for 256-step chunks.
  - One SBUF mega-tile holds [cstt | u blocks | state slots]: the first DMA
    transfer carries the stationary weights plus chunk 0, so a single
    DMA-complete semaphore gates the first real matmul (~10.7us; dynamic
    HWDGE rings cannot deliver data earlier).
  - Junk 64-col matmuls run continuously from the window start until the
    data lands: the DVFS governor (HAM) grants full PE clock only after
    ~4.5us of *uninterrupted* activity, and a gap resets the window.
  - psY is split per block group into separate PSUM pools (b0 / b1+b2) so
    bank recycling is fine-grained; DVE evacuates b1+b2 (emitted first),
    ACT the chain copy + b0.  PSUM: 2*(1+2+1) = 8 banks exactly.
  - All fp8 tensors are pre-scaled by powers of two (exactly compensated).
  - Fallback: bf16 variant (DoubleRow pairs split into two matmuls) if the
    conv part is not small relative to y.

Sharding: data-parallel over batch; each of 8 cores gets 512 sequences.
"""

import os
import sys

for _p in ("/opt/trn_rl_repo",):
    if _p not in sys.path and os.path.isdir(_p):
        sys.path.append(_p)

import numpy as np
import ml_dtypes

EPS = 1e-7          # complex_softmax eps
B, L, N = 4096, 4096, 64
N_CORES = 8
BC = B // N_CORES   # 512 sequences per core
P = 128             # partitions / block size
NBLK = L // P       # 32 blocks of 128 timesteps
CH = 3 * P          # 384-step full chunks
NCH = 11            # 10 full chunks + 1 final 2-block chunk
NJUNK = 10          # full-row warmup matmuls, bridge PE start -> data arrival
E4NP = ml_dtypes.float8_e4m3   # matches TRN FP8_EXP4 (max +-240)

# mv mega-tile slot map (slots of BC=512 fp8 bytes):
#   0..2   cstt (12 x [128,128] stationary ktiles, viewed as 3 slots)
#   3..34  u blocks: chunk c (c<10) at 3+3c..3+3c+2 = [u2, u1, u0];
#          last chunk at 33..34 = [u1, u0]
#   35..44 state S_c for chunk c=1..10 at slot 34+c
NSLOT = 45

_PROGS = {}         # compiled Bass programs, keyed by (use_fp8, yscale)


def _host_constants(Lambda_re, Lambda_im, W, D, log_step):
    """Block matrices for the chunked scan, in float64, plus scale exponents."""
    step = float(np.exp(np.float64(log_step[0])))
    Lam = Lambda_re.astype(np.float64) + 1j * Lambda_im.astype(np.float64)
    Wc = W[0, :, 0].astype(np.float64) + 1j * W[0, :, 1].astype(np.float64)
    s = np.arange(CH + 1, dtype=np.float64)
    pows = np.exp(np.outer(s, step * Lam))                      # (CH+1, N)
    Gamma = pows[CH]
    sl = np.arange(L, dtype=np.float64)
    powsL = np.exp(np.outer(sl, step * Lam))                    # (L, N)
    Sigma = powsL.sum(axis=0)
    wt = (Wc / Lam) * np.conj(Sigma) / (Sigma * np.conj(Sigma) + EPS)
    K = (pows[:CH] * wt[None, :]).sum(axis=1).real              # (CH,)
    Kfull = (powsL * wt[None, :]).sum(axis=1).real              # (L,)

    idx = np.arange(P)
    qp = idx[None, :] - idx[:, None]                            # q - p
    # T_k[s, q] = K[128k + q - s]: y block i += T_k @ u_{i-k}
    T0 = np.where(qp >= 0, K[np.clip(qp, 0, CH - 1)], 0.0)
    T1 = K[qp + P]
    T2 = K[qp + 2 * P]
    # A: state accumulation over the chunk; row = u timestep in chunk
    AP_ = pows[CH - 1 - np.arange(CH)]                          # (CH, N)
    AA = np.concatenate([AP_.real, AP_.imag], axis=1)           # (CH, 128)
    # M: one-chunk state transition (2x2 rotation blocks)
    MT = np.zeros((P, P))
    n = np.arange(N)
    MT[n, n] = Gamma.real
    MT[64 + n, n] = -Gamma.imag
    MT[n, 64 + n] = Gamma.imag
    MT[64 + n, 64 + n] = Gamma.real
    # V: state -> y within chunk; col = timestep offset in chunk
    Vq = pows[1:CH + 1] * wt[None, :]                           # (CH, N)
    VV = np.concatenate([Vq.real.T, -Vq.imag.T], axis=0)        # (128, CH)

    def pexp(target_max, cur_max):
        return int(np.floor(np.log2(target_max / max(cur_max, 1e-30))))

    tmax = max(np.abs(T0).max(), np.abs(T1).max(), np.abs(T2).max())
    eT = pexp(16.0, tmax)
    s_rms = np.sqrt((np.abs(pows[:CH]) ** 2).sum(axis=0)).max()  # state scale
    eS = pexp(1.0, s_rms)
    eV = eT - eS
    eA = eS
    k_rms = float(np.sqrt((Kfull ** 2).sum()))                  # ~ y_conv rms
    eY = pexp(8.0, 5.0 * k_rms)
    conv_share = k_rms / np.sqrt(float(D[0]) ** 2 + k_rms ** 2)
    return dict(T0=T0, T1=T1, T2=T2, AA=AA, MT=MT, VV=VV,
                eT=eT, eS=eS, eV=eV, eA=eA, eY=eY, conv_share=conv_share)


# cstt ktile layout (12 x [128, 128]):
#   0:T0 1:V0  2:T1 3:V1  4:T2 5:V2  6:T0 7:T1  8:A0 9:M  10:A2 11:A1
def _pack_consts(cs, np_dtype):
    T0, T1, T2 = cs["T0"], cs["T1"], cs["T2"]
    AA, MT, VV = cs["AA"], cs["MT"], cs["VV"]
    sT, sV, sA = 2.0 ** cs["eT"], 2.0 ** cs["eV"], 2.0 ** cs["eA"]
    cst = np.zeros((P, 12, P), dtype=np.float64)
    cst[:, 0] = T0 * sT
    cst[:, 1] = VV[:, 0:P] * sV
    cst[:, 2] = T1 * sT
    cst[:, 3] = VV[:, P:2 * P] * sV
    cst[:, 4] = T2 * sT
    cst[:, 5] = VV[:, 2 * P:3 * P] * sV
    cst[:, 6] = T0 * sT
    cst[:, 7] = T1 * sT
    cst[:, 8] = AA[0:P] * sA          # A0
    cst[:, 9] = MT                    # M (state units, unscaled)
    cst[:, 10] = AA[2 * P:3 * P] * sA  # A2
    cst[:, 11] = AA[P:2 * P] * sA      # A1
    return cst.astype(np_dtype)


def _build(use_fp8, yscale):
    import concourse.tile as tile
    from concourse import bacc, mybir
    from contextlib import ExitStack

    f32 = mybir.dt.float32
    mdt = mybir.dt.float8e4 if use_fp8 else mybir.dt.bfloat16
    DR = mybir.MatmulPerfMode.DoubleRow if use_fp8 else None

    nc = bacc.Bacc("TRN2", target_bir_lowering=False, debug=False,
                   num_devices=N_CORES)
    ut = nc.dram_tensor("ut8", [P, 35 * BC], mdt, kind="ExternalInput").ap()
    yt = nc.dram_tensor("yt8", [P, NBLK * BC], mdt, kind="ExternalOutput").ap()
    ut3 = ut.rearrange("p (k b) -> p k b", k=35)

    with tile.TileContext(nc) as tc, ExitStack() as ctx:
        cpool = ctx.enter_context(tc.tile_pool(name="const", bufs=1))
        mpool = ctx.enter_context(tc.tile_pool(name="mv", bufs=1))
        ypool = ctx.enter_context(tc.tile_pool(name="y", bufs=1))
        pb0 = ctx.enter_context(tc.tile_pool(name="pb0", bufs=2, space="PSUM"))
        pb1 = ctx.enter_context(tc.tile_pool(name="pb1", bufs=2, space="PSUM"))
        pb2 = ctx.enter_context(tc.tile_pool(name="pb2", bufs=2, space="PSUM"))
        pspool = ctx.enter_context(tc.tile_pool(name="pss", bufs=2, space="PSUM"))

        mv = mpool.tile([P, NSLOT, BC], mdt, tag="mv")
        yb = ypool.tile([P, NBLK * BC], mdt, tag="y")
        # stationary ktiles live in mv slots 0..2
        cstt = mv[:, 0:3, :].rearrange("p a (c d) -> p (a c) d", c=4)

        # DMA preloads (each dma_start is a ~0.65us serial DIRECT2D on SP):
        # first transfer = cstt + chunk 0 so one semaphore gates the first
        # matmul; then chunk 1, chunk 2, chunks 3-6, chunks 7-10.
        for lo, hi in ((0, 6), (6, 9), (9, 12), (12, 24), (24, 35)):
            nc.sync.dma_start(mv[:, lo:hi, :], ut3[:, lo:hi])

        # PE warmup: junk matmuls from window start until the data lands.
        # The DVFS governor picks the clock tier from sustained PE
        # *utilization* starting when PE activity begins (a gap resets the
        # window; low-utilization junk earns only a mid tier for the whole
        # run), so: full 128-row contraction, back-to-back, starting as
        # early as possible.  The operands are uninitialized yb garbage --
        # the junk result is never read, and the WAR edges (evacs write yb
        # much later) are free.
        psY0b0 = pb0.tile([P, BC], f32, tag="pb0", name="psb0_0")
        for _ in range(NJUNK):
            nc.tensor.matmul(psY0b0[:], yb[:, 0:128], yb[:, 128:640],
                             start=True, stop=True)

        cyscale = float(yscale)

        def mm(out_ap, cslot0, nslots, mv_ap, start, stop):
            """nslots=2: DoubleRow pair (two singles in bf16); nslots=1: single."""
            if nslots == 2 and use_fp8:
                nc.tensor.matmul(out_ap, cstt[:, cslot0:cslot0 + 2, :], mv_ap,
                                 start=start, stop=stop, perf_mode=DR)
            elif nslots == 2:
                nc.tensor.matmul(out_ap, cstt[:, cslot0, :], mv_ap[:, 0, :],
                                 start=start, stop=False)
                nc.tensor.matmul(out_ap, cstt[:, cslot0 + 1, :],
                                 mv_ap[:, 1, :], start=False, stop=stop)
            else:
                nc.tensor.matmul(out_ap, cstt[:, cslot0, :], mv_ap,
                                 start=start, stop=stop)

        for c in range(NCH):
            last = c == NCH - 1
            nb = 2 if last else 3          # blocks in this chunk
            u2, u1, u0 = 3 + 3 * c, 4 + 3 * c, 5 + 3 * c
            if last:
                u1, u0 = 33, 34
            ss = 34 + c                    # S_c slot (c >= 1)
            u21 = mv[:, u2:u2 + 2, :]
            u10 = mv[:, u1:u1 + 2, :]
            uS = mv[:, u0:ss + 1:ss - u0, :]   # (u0, S_c) 2-ktile view
            mu0, mu1 = mv[:, u0, :], mv[:, u1, :]

            psb0 = psY0b0 if c == 0 else pb0.tile([P, BC], f32, tag="pb0",
                                                  name=f"psb0_{c}")
            psb1 = pb1.tile([P, BC], f32, tag="pb1", name=f"psb1_{c}")
            psb2 = (pb2.tile([P, BC], f32, tag="pb2", name=f"psb2_{c}")
                    if not last else None)

            if not last:
                # state for chunk c+1 (chain-critical matmul emitted last)
                psS = pspool.tile([P, BC], f32, tag="pss", name=f"psS{c}")
                mm(psS[:], 10, 2, u21, True, False)     # A2*u2 + A1*u1
                if c == 0:
                    mm(psS[:], 8, 1, mu0, False, True)  # A0*u0 (no state yet)
                else:
                    mm(psS[:], 8, 2, uS, False, True)   # A0*u0 + M*S
                # chain copy: scaled state into chunk c+1's S slot
                nc.scalar.copy(mv[:, 34 + c + 1, :], psS[:])

            # psY blocks (block k of chunk c = global block 3c+k);
            # each block has its own PSUM tile, and its DVE evac is emitted
            # right after its matmuls so banks recycle per block.
            y0 = 3 * c * BC
            if c == 0:
                mm(psb0[:], 0, 1, mu0, True, True)           # T0*u0
                mm(psb1[:], 6, 2, u10, True, True)           # T0u1+T1u0
                nc.vector.tensor_scalar_mul(yb[:, y0 + BC:y0 + 2 * BC],
                                            psb1[:], cyscale)
                mm(psb2[:], 6, 2, u21, True, False)          # T0u2+T1u1
                mm(psb2[:], 4, 1, mu0, False, True)          # T2*u0
                nc.vector.tensor_scalar_mul(yb[:, y0 + 2 * BC:y0 + 3 * BC],
                                            psb2[:], cyscale)
            else:
                mm(psb0[:], 0, 2, uS, True, True)            # T0u0+V0S
                mm(psb1[:], 2, 2, uS, True, False)           # T1u0+V1S
                mm(psb1[:], 0, 1, mu1, False, True)          # T0*u1
                nc.vector.tensor_scalar_mul(yb[:, y0 + BC:y0 + 2 * BC],
                                            psb1[:], cyscale)
                if not last:
                    mm(psb2[:], 4, 2, uS, True, False)       # T2u0+V2S
                    mm(psb2[:], 6, 2, u21, False, True)      # T0u2+T1u1
                    nc.vector.tensor_scalar_mul(yb[:, y0 + 2 * BC:y0 + 3 * BC],
                                                psb2[:], cyscale)
            # ACT evacuates block 0 on top of the chain copy
            nc.scalar.mul(yb[:, y0:y0 + BC], psb0[:], cyscale)
            # y stores: batched early (SP DIRECT2D issue is ~0.65us serial),
            # per-chunk at the tail so the last store is small and early
            if c == 3:
                nc.sync.dma_start(yt[:, 0:12 * BC], yb[:, 0:12 * BC])
            elif c == 7:
                nc.sync.dma_start(yt[:, 12 * BC:24 * BC], yb[:, 12 * BC:24 * BC])
            elif c >= 8:
                nc.sync.dma_start(yt[:, y0:y0 + nb * BC], yb[:, y0:y0 + nb * BC])

    return nc


def _program(use_fp8, yscale):
    key = (use_fp8, yscale)
    if key not in _PROGS:
        nc = _build(use_fp8, yscale)
        nc.compile()
        _PROGS[key] = nc
    return _PROGS[key]


# Set PROFILE=True before calling kernel() to capture an NTFF profile;
# LAST_EXEC_NS then holds the measured hardware execution time.
PROFILE = False
LAST_EXEC_NS = None
LAST_RESULTS = None


def kernel(u, Lambda_re, Lambda_im, W, D, log_step):
    global LAST_EXEC_NS, LAST_RESULTS
    from concourse.bass_utils import run_bass_kernel_spmd

    u = np.asarray(u, dtype=np.float32)
    cs = _host_constants(np.asarray(Lambda_re), np.asarray(Lambda_im),
                         np.asarray(W), np.asarray(D), np.asarray(log_step))
    use_fp8 = cs["conv_share"] < 0.25
    np_dtype = E4NP if use_fp8 else ml_dtypes.bfloat16
    consts = _pack_consts(cs, np_dtype).reshape(P, 3, BC)  # 3 slots of BC
    scale = float(2.0 ** (cs["eY"] - cs["eT"]))
    nc = _program(use_fp8, scale)

    # dram u layout: slots 0..2 = cstt, slots 3..34 = u blocks with chunk
    # c's blocks stored reversed ([3c+2, 3c+1, 3c]; last chunk [31, 30])
    perm = []
    for c in range(10):
        perm.extend([3 * c + 2, 3 * c + 1, 3 * c])
    perm.extend([31, 30])
    perm = np.asarray(perm)

    in_maps = []
    for c in range(N_CORES):
        ush = u[c * BC:(c + 1) * BC, :].T                     # (L, BC)
        blocks = ush.reshape(NBLK, P, BC).transpose(1, 0, 2)  # (p, blk, b)
        ub = blocks[:, perm, :].astype(np_dtype)              # (p, 32, BC)
        arr = np.concatenate([consts, ub], axis=1).reshape(P, 35 * BC)
        in_maps.append({"ut8": np.ascontiguousarray(arr)})

    res = run_bass_kernel_spmd(nc, in_maps, list(range(N_CORES)), trace=PROFILE)
    if PROFILE:
        LAST_EXEC_NS = res.exec_time_ns
        LAST_RESULTS = res

    y = np.empty((B, L), dtype=np.float32)
    inv = np.float32(2.0 ** -cs["eY"])
    Df = np.float32(D[0])
    for c in range(N_CORES):
        y8 = res.results[c]["yt8"].reshape(P, NBLK, BC)       # (p, blk, b)
        yc = y8.astype(np.float32).transpose(1, 0, 2).reshape(L, BC)
        y[c * BC:(c + 1) * BC, :] = yc.T * inv + Df * u[c * BC:(c + 1) * BC, :]
    return y


# revision 17
# speedup vs baseline: 1.0199x; 1.0199x over previous
"""Trainium2 Bass kernel for the DSS (Diagonal State Space) layer.

y = irfft(rfft(u, 2L) * rfft(K, 2L))[:L] + D*u, with K the length-L DSS kernel
derived from (Lambda, W, log_step) via a complex softmax.

Strategy (v5, fp8, 384-step chunks):
  - The D*u term carries ~97% of the output energy for the reference params;
    it is added EXACTLY on the host.  The device computes only the small
    convolution part, which tolerates fp8 noise easily.
  - fp8 e4m3 on the wire: u in (2.1 MB/core), y_conv out (2.1 MB/core).
  - Chunked diagonal-SSM scan, time-major, 384-step chunks (3 blocks of 128):
    intra-chunk Toeplitz matmuls + rank-128 state (Re/Im of 64 modes).
    Per full chunk only 7 fp8 DoubleRow-packed matmuls (5 psY + 2 psS),
    using the (A0|M)@(u0|S) pairing so the state transition rides along
    with a u contraction.  72 matmuls total vs 80 for 256-step chunks.
  - One SBUF mega-tile holds [cstt | u blocks | state slots]: the first DMA
    transfer carries the stationary weights plus chunk 0, so a single
    DMA-complete semaphore gates the first real matmul (~10.7us; dynamic
    HWDGE rings cannot deliver data earlier).
  - Junk 64-col matmuls run continuously from the window start until the
    data lands: the DVFS governor (HAM) grants full PE clock only after
    ~4.5us of *uninterrupted* activity, and a gap resets the window.
  - psY is split per block group into separate PSUM pools (b0 / b1+b2) so
    bank recycling is fine-grained; DVE evacuates b1+b2 (emitted first),
    ACT the chain copy + b0.  PSUM: 2*(1+2+1) = 8 banks exactly.
  - All fp8 tensors are pre-scaled by powers of two (exactly compensated).
  - Fallback: bf16 variant (DoubleRow pairs split into two matmuls) if the
    conv part is not small relative to y.

Sharding: data-parallel over batch; each of 8 cores gets 512 sequences.
"""

import os
import sys

for _p in ("/opt/trn_rl_repo",):
    if _p not in sys.path and os.path.isdir(_p):
        sys.path.append(_p)

import numpy as np
import ml_dtypes

EPS = 1e-7          # complex_softmax eps
B, L, N = 4096, 4096, 64
N_CORES = 8
BC = B // N_CORES   # 512 sequences per core
P = 128             # partitions / block size
NBLK = L // P       # 32 blocks of 128 timesteps
CH = 3 * P          # 384-step full chunks
NCH = 11            # 10 full chunks + 1 final 2-block chunk
NJUNK = 10          # full-row warmup matmuls, bridge PE start -> data arrival
E4NP = ml_dtypes.float8_e4m3   # matches TRN FP8_EXP4 (max +-240)

# mv mega-tile slot map (slots of BC=512 fp8 bytes):
#   0..2   cstt (12 x [128,128] stationary ktiles, viewed as 3 slots)
#   3..34  u blocks: chunk c (c<10) at 3+3c..3+3c+2 = [u2, u1, u0];
#          last chunk at 33..34 = [u1, u0]
#   35..44 state S_c for chunk c=1..10 at slot 34+c
NSLOT = 45

_PROGS = {}         # compiled Bass programs, keyed by (use_fp8, yscale)


def _host_constants(Lambda_re, Lambda_im, W, D, log_step):
    """Block matrices for the chunked scan, in float64, plus scale exponents."""
    step = float(np.exp(np.float64(log_step[0])))
    Lam = Lambda_re.astype(np.float64) + 1j * Lambda_im.astype(np.float64)
    Wc = W[0, :, 0].astype(np.float64) + 1j * W[0, :, 1].astype(np.float64)
    s = np.arange(CH + 1, dtype=np.float64)
    pows = np.exp(np.outer(s, step * Lam))                      # (CH+1, N)
    Gamma = pows[CH]
    sl = np.arange(L, dtype=np.float64)
    powsL = np.exp(np.outer(sl, step * Lam))                    # (L, N)
    Sigma = powsL.sum(axis=0)
    wt = (Wc / Lam) * np.conj(Sigma) / (Sigma * np.conj(Sigma) + EPS)
    K = (pows[:CH] * wt[None, :]).sum(axis=1).real              # (CH,)
    Kfull = (powsL * wt[None, :]).sum(axis=1).real              # (L,)

    idx = np.arange(P)
    qp = idx[None, :] - idx[:, None]                            # q - p
    # T_k[s, q] = K[128k + q - s]: y block i += T_k @ u_{i-k}
    T0 = np.where(qp >= 0, K[np.clip(qp, 0, CH - 1)], 0.0)
    T1 = K[qp + P]
    T2 = K[qp + 2 * P]
    # A: state accumulation over the chunk; row = u timestep in chunk
    AP_ = pows[CH - 1 - np.arange(CH)]                          # (CH, N)
    AA = np.concatenate([AP_.real, AP_.imag], axis=1)           # (CH, 128)
    # M: one-chunk state transition (2x2 rotation blocks)
    MT = np.zeros((P, P))
    n = np.arange(N)
    MT[n, n] = Gamma.real
    MT[64 + n, n] = -Gamma.imag
    MT[n, 64 + n] = Gamma.imag
    MT[64 + n, 64 + n] = Gamma.real
    # V: state -> y within chunk; col = timestep offset in chunk
    Vq = pows[1:CH + 1] * wt[None, :]                           # (CH, N)
    VV = np.concatenate([Vq.real.T, -Vq.imag.T], axis=0)        # (128, CH)

    def pexp(target_max, cur_max):
        return int(np.floor(np.log2(target_max / max(cur_max, 1e-30))))

    tmax = max(np.abs(T0).max(), np.abs(T1).max(), np.abs(T2).max())
    eT = pexp(16.0, tmax)
    s_rms = np.sqrt((np.abs(pows[:CH]) ** 2).sum(axis=0)).max()  # state scale
    eS = pexp(1.0, s_rms)
    eV = eT - eS
    eA = eS
    k_rms = float(np.sqrt((Kfull ** 2).sum()))                  # ~ y_conv rms
    eY = pexp(8.0, 5.0 * k_rms)
    conv_share = k_rms / np.sqrt(float(D[0]) ** 2 + k_rms ** 2)
    return dict(T0=T0, T1=T1, T2=T2, AA=AA, MT=MT, VV=VV,
                eT=eT, eS=eS, eV=eV, eA=eA, eY=eY, conv_share=conv_share)


# cstt ktile layout (12 x [128, 128]):
#   0:T0 1:V0  2:T1 3:V1  4:T2 5:V2  6:T0 7:T1  8:A0 9:M  10:A2 11:A1
def _pack_consts(cs, np_dtype):
    T0, T1, T2 = cs["T0"], cs["T1"], cs["T2"]
    AA, MT, VV = cs["AA"], cs["MT"], cs["VV"]
    sT, sV, sA = 2.0 ** cs["eT"], 2.0 ** cs["eV"], 2.0 ** cs["eA"]
    cst = np.zeros((P, 12, P), dtype=np.float64)
    cst[:, 0] = T0 * sT
    cst[:, 1] = VV[:, 0:P] * sV
    cst[:, 2] = T1 * sT
    cst[:, 3] = VV[:, P:2 * P] * sV
    cst[:, 4] = T2 * sT
    cst[:, 5] = VV[:, 2 * P:3 * P] * sV
    cst[:, 6] = T0 * sT
    cst[:, 7] = T1 * sT
    cst[:, 8] = AA[0:P] * sA          # A0
    cst[:, 9] = MT                    # M (state units, unscaled)
    cst[:, 10] = AA[2 * P:3 * P] * sA  # A2
    cst[:, 11] = AA[P:2 * P] * sA      # A1
    return cst.astype(np_dtype)


def _build(use_fp8, yscale):
    import concourse.tile as tile
    from concourse import bacc, mybir
    from contextlib import ExitStack

    f32 = mybir.dt.float32
    mdt = mybir.dt.float8e4 if use_fp8 else mybir.dt.bfloat16
    DR = mybir.MatmulPerfMode.DoubleRow if use_fp8 else None

    nc = bacc.Bacc("TRN2", target_bir_lowering=False, debug=False,
                   num_devices=N_CORES)
    ut = nc.dram_tensor("ut8", [P, 35 * BC], mdt, kind="ExternalInput").ap()
    yt = nc.dram_tensor("yt8", [P, NBLK * BC], mdt, kind="ExternalOutput").ap()
    ut3 = ut.rearrange("p (k b) -> p k b", k=35)

    with tile.TileContext(nc) as tc, ExitStack() as ctx:
        cpool = ctx.enter_context(tc.tile_pool(name="const", bufs=1))
        mpool = ctx.enter_context(tc.tile_pool(name="mv", bufs=1))
        ypool = ctx.enter_context(tc.tile_pool(name="y", bufs=1))
        pb0 = ctx.enter_context(tc.tile_pool(name="pb0", bufs=2, space="PSUM"))
        pb1 = ctx.enter_context(tc.tile_pool(name="pb1", bufs=2, space="PSUM"))
        pb2 = ctx.enter_context(tc.tile_pool(name="pb2", bufs=2, space="PSUM"))
        pspool = ctx.enter_context(tc.tile_pool(name="pss", bufs=2, space="PSUM"))

        mv = mpool.tile([P, NSLOT, BC], mdt, tag="mv")
        yb = ypool.tile([P, NBLK * BC], mdt, tag="y")
        # stationary ktiles live in mv slots 0..2
        cstt = mv[:, 0:3, :].rearrange("p a (c d) -> p (a c) d", c=4)

        # DMA preloads (each dma_start is a ~0.65us serial DIRECT2D on SP):
        # first transfer = cstt + chunk 0 so one semaphore gates the first
        # matmul; then chunk 1, chunk 2, chunks 3-6, chunks 7-10.
        for lo, hi in ((0, 6), (6, 9), (9, 12), (12, 15), (15, 18),
                       (18, 24), (24, 35)):
            nc.sync.dma_start(mv[:, lo:hi, :], ut3[:, lo:hi])

        # PE warmup: junk matmuls from window start until the data lands.
        # The DVFS governor picks the clock tier from sustained PE
        # *utilization* starting when PE activity begins (a gap resets the
        # window; low-utilization junk earns only a mid tier for the whole
        # run), so: full 128-row contraction, back-to-back, starting as
        # early as possible.  The operands are uninitialized yb garbage --
        # the junk result is never read, and the WAR edges (evacs write yb
        # much later) are free.
        psY0b0 = pb0.tile([P, BC], f32, tag="pb0", name="psb0_0")
        for _ in range(NJUNK):
            nc.tensor.matmul(psY0b0[:], yb[:, 0:128], yb[:, 128:640],
                             start=True, stop=True)

        cyscale = float(yscale)

        def mm(out_ap, cslot0, nslots, mv_ap, start, stop):
            """nslots=2: DoubleRow pair (two singles in bf16); nslots=1: single."""
            if nslots == 2 and use_fp8:
                nc.tensor.matmul(out_ap, cstt[:, cslot0:cslot0 + 2, :], mv_ap,
                                 start=start, stop=stop, perf_mode=DR)
            elif nslots == 2:
                nc.tensor.matmul(out_ap, cstt[:, cslot0, :], mv_ap[:, 0, :],
                                 start=start, stop=False)
                nc.tensor.matmul(out_ap, cstt[:, cslot0 + 1, :],
                                 mv_ap[:, 1, :], start=False, stop=stop)
            else:
                nc.tensor.matmul(out_ap, cstt[:, cslot0, :], mv_ap,
                                 start=start, stop=stop)

        for c in range(NCH):
            last = c == NCH - 1
            nb = 2 if last else 3          # blocks in this chunk
            u2, u1, u0 = 3 + 3 * c, 4 + 3 * c, 5 + 3 * c
            if last:
                u1, u0 = 33, 34
            ss = 34 + c                    # S_c slot (c >= 1)
            u21 = mv[:, u2:u2 + 2, :]
            u10 = mv[:, u1:u1 + 2, :]
            uS = mv[:, u0:ss + 1:ss - u0, :]   # (u0, S_c) 2-ktile view
            mu0, mu1 = mv[:, u0, :], mv[:, u1, :]

            psb0 = psY0b0 if c == 0 else pb0.tile([P, BC], f32, tag="pb0",
                                                  name=f"psb0_{c}")
            psb1 = pb1.tile([P, BC], f32, tag="pb1", name=f"psb1_{c}")
            psb2 = (pb2.tile([P, BC], f32, tag="pb2", name=f"psb2_{c}")
                    if not last else None)

            if not last:
                # state for chunk c+1 (chain-critical matmul emitted last)
                psS = pspool.tile([P, BC], f32, tag="pss", name=f"psS{c}")
                mm(psS[:], 10, 2, u21, True, False)     # A2*u2 + A1*u1
                if c == 0:
                    mm(psS[:], 8, 1, mu0, False, True)  # A0*u0 (no state yet)
                else:
                    mm(psS[:], 8, 2, uS, False, True)   # A0*u0 + M*S
                # chain copy: scaled state into chunk c+1's S slot
                nc.scalar.copy(mv[:, 34 + c + 1, :], psS[:])

            # psY blocks (block k of chunk c = global block 3c+k);
            # each block has its own PSUM tile, and its DVE evac is emitted
            # right after its matmuls so banks recycle per block.
            y0 = 3 * c * BC
            if c == 0:
                mm(psb0[:], 0, 1, mu0, True, True)           # T0*u0
                mm(psb1[:], 6, 2, u10, True, True)           # T0u1+T1u0
                nc.vector.tensor_scalar_mul(yb[:, y0 + BC:y0 + 2 * BC],
                                            psb1[:], cyscale)
                mm(psb2[:], 6, 2, u21, True, False)          # T0u2+T1u1
                mm(psb2[:], 4, 1, mu0, False, True)          # T2*u0
                nc.vector.tensor_scalar_mul(yb[:, y0 + 2 * BC:y0 + 3 * BC],
                                            psb2[:], cyscale)
            else:
                mm(psb0[:], 0, 2, uS, True, True)            # T0u0+V0S
                mm(psb1[:], 2, 2, uS, True, False)           # T1u0+V1S
                mm(psb1[:], 0, 1, mu1, False, True)          # T0*u1
                nc.vector.tensor_scalar_mul(yb[:, y0 + BC:y0 + 2 * BC],
                                            psb1[:], cyscale)
                if not last:
                    mm(psb2[:], 4, 2, uS, True, False)       # T2u0+V2S
                    mm(psb2[:], 6, 2, u21, False, True)      # T0u2+T1u1
                    nc.vector.tensor_scalar_mul(yb[:, y0 + 2 * BC:y0 + 3 * BC],
                                                psb2[:], cyscale)
            # ACT evacuates block 0 on top of the chain copy
            nc.scalar.mul(yb[:, y0:y0 + BC], psb0[:], cyscale)
            # y stores: batched early (SP DIRECT2D issue is ~0.65us serial),
            # per-chunk at the tail so the last store is small and early
            if c == 3:
                nc.sync.dma_start(yt[:, 0:12 * BC], yb[:, 0:12 * BC])
            elif c == 7:
                nc.sync.dma_start(yt[:, 12 * BC:24 * BC], yb[:, 12 * BC:24 * BC])
            elif c >= 8:
                nc.sync.dma_start(yt[:, y0:y0 + nb * BC], yb[:, y0:y0 + nb * BC])

    return nc


def _program(use_fp8, yscale):
    key = (use_fp8, yscale)
    if key not in _PROGS:
        nc = _build(use_fp8, yscale)
        nc.compile()
        _PROGS[key] = nc
    return _PROGS[key]


# Set PROFILE=True before calling kernel() to capture an NTFF profile;
# LAST_EXEC_NS then holds the measured hardware execution time.
PROFILE = False
LAST_EXEC_NS = None
LAST_RESULTS = None


def kernel(u, Lambda_re, Lambda_im, W, D, log_step):
    global LAST_EXEC_NS, LAST_RESULTS
    from concourse.bass_utils import run_bass_kernel_spmd

    u = np.asarray(u, dtype=np.float32)
    cs = _host_constants(np.asarray(Lambda_re), np.asarray(Lambda_im),
                         np.asarray(W), np.asarray(D), np.asarray(log_step))
    use_fp8 = cs["conv_share"] < 0.25
    np_dtype = E4NP if use_fp8 else ml_dtypes.bfloat16
    consts = _pack_consts(cs, np_dtype).reshape(P, 3, BC)  # 3 slots of BC
    scale = float(2.0 ** (cs["eY"] - cs["eT"]))
    nc = _program(use_fp8, scale)

    # dram u layout: slots 0..2 = cstt, slots 3..34 = u blocks with chunk
    # c's blocks stored reversed ([3c+2, 3c+1, 3c]; last chunk [31, 30])
    perm = []
    for c in range(10):
        perm.extend([3 * c + 2, 3 * c + 1, 3 * c])
    perm.extend([31, 30])
    perm = np.asarray(perm)

    in_maps = []
    for c in range(N_CORES):
        ush = u[c * BC:(c + 1) * BC, :].T                     # (L, BC)
        blocks = ush.reshape(NBLK, P, BC).transpose(1, 0, 2)  # (p, blk, b)
        ub = blocks[:, perm, :].astype(np_dtype)              # (p, 32, BC)
        arr = np.concatenate([consts, ub], axis=1).reshape(P, 35 * BC)
        in_maps.append({"ut8": np.ascontiguousarray(arr)})

    res = run_bass_kernel_spmd(nc, in_maps, list(range(N_CORES)), trace=PROFILE)
    if PROFILE:
        LAST_EXEC_NS = res.exec_time_ns
        LAST_RESULTS = res

    y = np.empty((B, L), dtype=np.float32)
    inv = np.float32(2.0 ** -cs["eY"])
    Df = np.float32(D[0])
    for c in range(N_CORES):
        y8 = res.results[c]["yt8"].reshape(P, NBLK, BC)       # (p, blk, b)
        yc = y8.astype(np.float32).transpose(1, 0, 2).reshape(L, BC)
        y[c * BC:(c + 1) * BC, :] = yc.T * inv + Df * u[c * BC:(c + 1) * BC, :]
    return y
